# revision 25
# baseline (speedup 1.0000x reference)
"""AdaptiveCategoryMSA Trainium2 kernel (8 NeuronCores, data-parallel).

Host: category argmax + stable argsort; gather + logit-scale fold + fp16
pack. Device (per core = one batch-half = 64 groups of 128 tokens):
fp16 S matmuls (f32 psum), DVE rowmax (negate -> exp bias), 8 per-head
biased Act exps -> E fp16 sbuf, 8 PE transposes -> ET psum fp16, one DVE
mega-copy -> ET sbuf, 8 Y matmuls with ones-column denominators, DVE
reciprocal, Y normalize, 2 YT transposes + copy, 2 proj matmuls, Act
outcopy, batched DMAs (4 groups per DMA). Software-pipelined with a
1-group lag so all five engines stream.
Sharding: core cx = 2*b + half handles batch b, tokens [8192*half, ...).
"""
import sys
sys.path.insert(0, "/opt/trn_rl_repo")
import numpy as np

import concourse.bass as bass
import concourse.bacc as bacc
import concourse.mybir as mybir
from concourse.tile import TileContext
from concourse.bass_utils import run_bass_kernel_spmd

F32 = mybir.dt.float32
F16 = mybir.dt.float16

NG = 64          # groups per core
GB = 4           # groups per DMA batch
W = 776          # per-group input width: qk 512 + v(+ones) 264
C = 256

_cache = {}
_last_in_maps = None

# op placement toggles (tuned against the timeline cost model)
YNORM_BCAST = True      # one tensor_tensor with broadcast rinv
YNORM_ENGINE = "vector"   # pool | vector
YTCOPY_ENGINE = "vector"  # pool | vector | scalar
OUTCOPY_ENGINE = "vector"
MSUB_PE = True          # subtract rowmax via PE one-hot matmul + single exp
ETCOPY_SPLIT = 0.5      # fraction of ET copy on scalar engine (rest on vector)


def _eng(nc, name):
    return {"pool": nc.gpsimd, "vector": nc.vector, "scalar": nc.scalar}[name]


def _copy(nc, name, out, in_):
    if name == "scalar":
        nc.scalar.copy(out, in_)
    else:
        _eng(nc, name).tensor_copy(out, in_)


def _build(with_bias: bool):
    nc = bacc.Bacc(
        "TRN2", target_bir_lowering=False, debug=False,
        enable_asserts=True, num_devices=8,
    )
    qvd = nc.dram_tensor("qvd", [NG // GB, GB, 128, W], F16, kind="ExternalInput")
    wtd = nc.dram_tensor("wtd", [128, 2 * C], F16, kind="ExternalInput")
    idmd = nc.dram_tensor("idmd", [128, 128], F16, kind="ExternalInput")
    if with_bias:
        biasd = nc.dram_tensor("biasd", [1, C], F16, kind="ExternalInput")
    outd = nc.dram_tensor("outd", [NG // GB, GB, 128, C], F16, kind="ExternalOutput")

    AX = mybir.AxisListType.X
    EXP = mybir.ActivationFunctionType.Exp

    with TileContext(nc) as tc:
        with tc.tile_pool(name="const", bufs=1) as cpool, \
             tc.tile_pool(name="sbin", bufs=2) as sbin, \
             tc.tile_pool(name="sbe", bufs=2) as sbe, \
             tc.tile_pool(name="sbsm", bufs=2) as sbsm, \
             tc.tile_pool(name="sbo", bufs=2) as sbo, \
             tc.tile_pool(name="psS", bufs=3, space="PSUM") as psS, \
             tc.tile_pool(name="psY", bufs=1, space="PSUM") as psY, \
             tc.tile_pool(name="psO", bufs=1, space="PSUM") as psO:

            wt_sb = cpool.tile([128, 2 * C], F16)
            nc.sync.dma_start(wt_sb[:, :], wtd[:, :])
            idm = cpool.tile([128, 128], F16)
            nc.sync.dma_start(idm[:, :], idmd[:, :])
            ones1_sb = cpool.tile([1, 128], F16)
            nc.gpsimd.memset(ones1_sb[:, :], 1.0)
            if with_bias:
                bias_sb = cpool.tile([1, C], F16)
                nc.sync.dma_start(bias_sb[:, :], biasd[:, :])


            shared = {}

            def fetch(b):
                if b >= NG // GB:
                    return
                qv = sbin.tile([128, GB * W], F16, tag="qv", name=f"qv_{b}")
                nc.sync.dma_start(
                    qv.rearrange("p (g j) -> p g j", g=GB),
                    qvd[b, :, :, :].rearrange("g p j -> p g j"))
                shared[f"qv{b}"] = qv

            def stage_s(g, st):
                """PE: S matmuls (accumulation group left open for msub)."""
                if g % GB == 0:
                    fetch(g // GB + 1)   # prefetch next batch (batch 0 pre-issued)
                qv = shared[f"qv{g // GB}"]
                off = (g % GB) * W
                smega = psS.tile([128, 1024], F32, tag="s")
                for h in range(8):
                    c, hm = h // 4, h % 4
                    qs = qv[32 * hm:32 * hm + 32, off + 128 * c: off + 128 * c + 128]
                    ks = qv[32 * hm:32 * hm + 32,
                            off + 256 + 128 * c: off + 256 + 128 * c + 128]
                    tp = (96, 0) if hm == 3 else None
                    nc.tensor.matmul(smega[:, 128 * h:128 * h + 128], ks, qs,
                                     start=True, stop=False,
                                     tile_position=tp, skip_group_check=True)
                st.update(qv=qv, off=off, smega=smega, g=g)

            def stage_red(g, st):
                """Pool: negated per-(head,q) colmax of S^T via partition reduce."""
                smega = st["smega"]
                negm = sbsm.tile([1, 1024], F16, tag="negm")
                nc.gpsimd.tensor_reduce(
                    negm[:, :], smega[:, :],
                    axis=mybir.AxisListType.C, op=mybir.AluOpType.max,
                    negate=True)
                st["negm"] = negm

            def stage_msub(g, st):
                """PE: K=1 matmul broadcasts -rowmax into every S^T row."""
                nc.tensor.matmul(st["smega"][:, :], ones1_sb[:, :],
                                 st.pop("negm")[:, :],
                                 start=False, stop=True, skip_group_check=True)

            def stage_exp(g, st):
                """Act: single unbiased exp; output IS E^T (k-major)."""
                esb = sbe.tile([128, 1024], F16, tag="esb")
                nc.scalar.activation(esb[:, :], st["smega"][:, :], EXP,
                                     bias=0.0, scale=1.0)
                st["etsb"] = esb

            def stage_y(g, st):
                qv, off, etsb = st["qv"], st["off"], st["etsb"]
                yp = psY.tile([128, 264], F32, tag="y")
                for h in range(8):
                    nc.tensor.matmul(
                        yp[:, 33 * h:33 * h + 33],
                        etsb[:, 128 * h:128 * h + 128],
                        qv[:, off + 512 + 33 * h: off + 512 + 33 * h + 33],
                        start=True, stop=True)
                rinv = sbsm.tile([128, 8], F32, tag="rinv")
                y3 = yp.rearrange("p (h j) -> p h j", h=8)
                nc.vector.reciprocal(rinv.rearrange("p (h j) -> p h j", j=1),
                                     y3[:, :, 32:33])
                st.update(yp=yp, rinv=rinv)

            def stage_ynorm(g, st):
                yp, rinv = st.pop("yp"), st.pop("rinv")
                y3 = yp.rearrange("p (h j) -> p h j", h=8)
                ysb = sbsm.tile([128, 256], F16, tag="ysb")
                _eng(nc, YNORM_ENGINE).tensor_tensor(
                    ysb.rearrange("p (h j) -> p h j", h=8),
                    y3[:, :, 0:32],
                    rinv.rearrange("p (h j) -> p h j", j=1).broadcast_to([128, 8, 32]),
                    op=mybir.AluOpType.mult)
                st["ysb"] = ysb

            def stage_yt(g, st):
                ysb = st.pop("ysb")
                ytp = psY.tile([128, 256], F16, tag="y", name="ytp")
                for ck in range(2):
                    nc.tensor.transpose(ytp[:, 128 * ck:128 * ck + 128],
                                        ysb[:, 128 * ck:128 * ck + 128], idm[:, :])
                ytsb = sbsm.tile([128, 256], F16, tag="ytsb")
                _copy(nc, YTCOPY_ENGINE, ytsb[:, :], ytp[:, :])
                st["ytsb"] = ytsb

            def stage_proj(g, st):
                ytsb = st.pop("ytsb")
                op = psO.tile([128, C], F32, tag="o")
                if with_bias:
                    nc.tensor.matmul(op[:, :], ones1_sb[:, :], bias_sb[:, :],
                                     start=True, stop=False)
                nc.tensor.matmul(op[:, :], ytsb[:, 0:128], wt_sb[:, 0:C],
                                 start=not with_bias, stop=False)
                nc.tensor.matmul(op[:, :], ytsb[:, 128:256], wt_sb[:, C:2 * C],
                                 start=False, stop=True)
                if g % GB == 0:
                    shared["osb"] = sbo.tile([128, GB * C], F16, tag="osb",
                                             name="osb")
                osb = shared["osb"]
                gi = g % GB
                _copy(nc, OUTCOPY_ENGINE, osb[:, C * gi:C * gi + C], op[:, :])
                if gi == GB - 1:
                    nc.sync.dma_start(
                        outd[g // GB, :, :, :].rearrange("g p j -> p g j"),
                        osb.rearrange("p (g j) -> p g j", g=GB))

            fetch(0)
            sts = {}
            LAG = 4
            for it in range(NG + LAG):
                def live(k):
                    return 0 <= k < NG

                if live(it):
                    sts[it] = {}
                    stage_s(it, sts[it])                 # PE: S first
                if live(it - 1):
                    stage_msub(it - 1, sts[it - 1])      # PE
                if live(it - 4):
                    stage_proj(it - 4, sts[it - 4])      # PE; outcopy
                if live(it):
                    stage_red(it, sts[it])               # Pool: red
                if live(it - 1):
                    stage_exp(it - 1, sts[it - 1])       # Act
                if live(it - 2):
                    stage_y(it - 2, sts[it - 2])         # PE Y; DVE recip
                    stage_ynorm(it - 2, sts[it - 2])     # DVE/Pool
                if live(it - 3):
                    stage_yt(it - 3, sts[it - 3])        # PE YT; YTcopy
                if live(it - 4):
                    sts.pop(it - 4)

    nc.finalize()
    return nc


def _prep_inputs(qkv, sim, proj_w, proj_b, scale):
    """Host-side shard + pack. Returns (in_maps, sort_indices, with_bias)."""
    b, n, _ = qkv.shape
    tk = np.argmax(sim, axis=-1)
    sort_idx = np.argsort(tk, axis=-1, kind="stable")

    wt_full = np.ascontiguousarray(proj_w.T).astype(np.float16)   # [cin, cout]
    with_bias = bool(np.any(proj_b != 0))
    bias16 = proj_b.reshape(1, C).astype(np.float16)
    idm = np.eye(128, dtype=np.float16)

    in_maps = []
    for cx in range(8):
        bi, half = cx // 2, cx % 2
        perm = sort_idx[bi, 8192 * half:8192 * (half + 1)]
        shuf = qkv[bi][perm].astype(np.float32)                    # [8192, 768]
        qk = shuf[:, 0:512].copy()
        qk[:, 0:256] *= scale
        # [g, tok, 4, 128] -> [g, p=ch, c, tok]
        qkt = qk.astype(np.float16).reshape(NG, 128, 4, 128).transpose(0, 3, 2, 1)
        qkt = qkt.reshape(NG, 128, 512)
        vpart = np.empty((NG, 128, 8, 33), dtype=np.float16)
        vpart[:, :, :, 0:32] = shuf[:, 512:768].reshape(NG, 128, 8, 32)
        vpart[:, :, :, 32] = 1.0
        qv = np.concatenate([qkt, vpart.reshape(NG, 128, 264)], axis=2)
        qv = np.ascontiguousarray(qv.reshape(NG // GB, GB, 128, W))
        m = {"qvd": qv, "wtd": wt_full, "idmd": idm}
        if with_bias:
            m["biasd"] = bias16
        in_maps.append(m)
    return in_maps, sort_idx, with_bias


def kernel(qkv, sim, proj_w, proj_b, logit_scale, h=128, w=128, **_unused):
    qkv = np.ascontiguousarray(np.asarray(qkv, dtype=np.float32))
    sim = np.asarray(sim, dtype=np.float32)
    proj_w = np.asarray(proj_w, dtype=np.float32)
    proj_b = np.asarray(proj_b, dtype=np.float32)
    ls = float(np.asarray(logit_scale, dtype=np.float32).reshape(-1)[0])
    scale = float(np.exp(min(ls, float(np.log(100.0)))))

    b, n, c3 = qkv.shape
    assert (b, n, c3) == (4, 16384, 768)

    in_maps, sort_idx, with_bias = _prep_inputs(qkv, sim, proj_w, proj_b, scale)

    key = ("b" if with_bias else "nb")
    if key not in _cache:
        _cache[key] = _build(with_bias)
    nc = _cache[key]

    global _last_in_maps
    _last_in_maps = in_maps
    res = run_bass_kernel_spmd(nc, in_maps, core_ids=list(range(8)))

    outf = np.empty((4, 16384, 256), dtype=np.float32)
    for cx in range(8):
        bi, half = cx // 2, cx % 2
        perm = sort_idx[bi, 8192 * half:8192 * (half + 1)]
        y = np.asarray(res.results[cx]["outd"]).astype(np.float32).reshape(8192, 256)
        outf[bi][perm] = y
    return outf


if __name__ == "__main__":
    rng = np.random.default_rng(0)
    qkv = rng.standard_normal((4, 16384, 768), dtype=np.float32)
    sim = rng.standard_normal((4, 16384, 64), dtype=np.float32)
    pw = (rng.standard_normal((256, 256), dtype=np.float32) * 0.02)
    pb = np.zeros(256, dtype=np.float32)
    lsc = np.log(10.0 * np.ones((1, 1), dtype=np.float32))
    o = kernel(qkv=qkv, sim=sim, proj_w=pw, proj_b=pb, logit_scale=lsc)
    print("ran", o.shape, o.dtype)


# revision 26
# speedup vs baseline: 1.2618x; 1.2618x over previous
"""AdaptiveCategoryMSA Trainium2 kernel (8 NeuronCores, data-parallel).

Host: category argmax + stable argsort; gather + logit-scale fold + fp16
pack. Device (per core = one batch-half = 64 groups of 128 tokens):
fp16 S matmuls (f32 psum), DVE rowmax (negate -> exp bias), 8 per-head
biased Act exps -> E fp16 sbuf, 8 PE transposes -> ET psum fp16, one DVE
mega-copy -> ET sbuf, 8 Y matmuls with ones-column denominators, DVE
reciprocal, Y normalize, 2 YT transposes + copy, 2 proj matmuls, Act
outcopy, batched DMAs (4 groups per DMA). Software-pipelined with a
1-group lag so all five engines stream.
Sharding: core cx = 2*b + half handles batch b, tokens [8192*half, ...).
"""
import sys
sys.path.insert(0, "/opt/trn_rl_repo")
import numpy as np

import concourse.bass as bass
import concourse.bacc as bacc
import concourse.mybir as mybir
from concourse.tile import TileContext
from concourse.bass_utils import run_bass_kernel_spmd

F32 = mybir.dt.float32
F16 = mybir.dt.float16

NG = 64          # groups per core
GB = 4           # groups per DMA batch
W = 776          # per-group input width: qk 512 + v(+ones) 264
C = 256

_cache = {}
_last_in_maps = None

# op placement toggles (tuned against the timeline cost model)
YNORM_BCAST = True      # one tensor_tensor with broadcast rinv
YNORM_ENGINE = "vector"   # pool | vector
YTCOPY_ENGINE = "vector"  # pool | vector | scalar
OUTCOPY_ENGINE = "vector"
MSUB_PE = True          # subtract rowmax via PE one-hot matmul + single exp
ETCOPY_SPLIT = 0.5      # fraction of ET copy on scalar engine (rest on vector)


def _eng(nc, name):
    return {"pool": nc.gpsimd, "vector": nc.vector, "scalar": nc.scalar}[name]


def _copy(nc, name, out, in_):
    if name == "scalar":
        nc.scalar.copy(out, in_)
    else:
        _eng(nc, name).tensor_copy(out, in_)


def _build(with_bias: bool):
    nc = bacc.Bacc(
        "TRN2", target_bir_lowering=False, debug=False,
        enable_asserts=True, num_devices=8,
    )
    qvd = nc.dram_tensor("qvd", [NG // GB, GB, 128, W], F16, kind="ExternalInput")
    wtd = nc.dram_tensor("wtd", [128, 2 * C], F16, kind="ExternalInput")
    idmd = nc.dram_tensor("idmd", [128, 128], F16, kind="ExternalInput")
    if with_bias:
        biasd = nc.dram_tensor("biasd", [1, C], F16, kind="ExternalInput")
    outd = nc.dram_tensor("outd", [NG // GB, GB, 128, C], F16, kind="ExternalOutput")

    AX = mybir.AxisListType.X
    EXP = mybir.ActivationFunctionType.Exp

    with TileContext(nc) as tc:
        with tc.tile_pool(name="const", bufs=1) as cpool, \
             tc.tile_pool(name="sbin", bufs=2) as sbin, \
             tc.tile_pool(name="sbe", bufs=2) as sbe, \
             tc.tile_pool(name="sbsm", bufs=2) as sbsm, \
             tc.tile_pool(name="sbo", bufs=2) as sbo, \
             tc.tile_pool(name="psS", bufs=3, space="PSUM") as psS, \
             tc.tile_pool(name="psY", bufs=1, space="PSUM") as psY, \
             tc.tile_pool(name="psO", bufs=1, space="PSUM") as psO:

            wt_sb = cpool.tile([128, 2 * C], F16)
            nc.sync.dma_start(wt_sb[:, :], wtd[:, :])
            idm = cpool.tile([128, 128], F16)
            nc.sync.dma_start(idm[:, :], idmd[:, :])
            ones1_sb = cpool.tile([1, 128], F16)
            nc.gpsimd.memset(ones1_sb[:, :], 1.0)
            if with_bias:
                bias_sb = cpool.tile([1, C], F16)
                nc.sync.dma_start(bias_sb[:, :], biasd[:, :])


            shared = {}

            def fetch(b):
                if b >= NG // GB:
                    return
                qv = sbin.tile([128, GB * W], F16, tag="qv", name=f"qv_{b}")
                nc.sync.dma_start(
                    qv.rearrange("p (g j) -> p g j", g=GB),
                    qvd[b, :, :, :].rearrange("g p j -> p g j"))
                shared[f"qv{b}"] = qv

            def stage_s(g, st):
                """PE: S matmuls (accumulation group left open for msub)."""
                if g % GB == 0:
                    fetch(g // GB + 1)   # prefetch next batch (batch 0 pre-issued)
                qv = shared[f"qv{g // GB}"]
                off = (g % GB) * W
                smega = psS.tile([128, 1024], F32, tag="s")
                for h in range(8):
                    c, hm = h // 4, h % 4
                    qs = qv[32 * hm:32 * hm + 32, off + 128 * c: off + 128 * c + 128]
                    ks = qv[32 * hm:32 * hm + 32,
                            off + 256 + 128 * c: off + 256 + 128 * c + 128]
                    tp = (96, 0) if hm == 3 else None
                    nc.tensor.matmul(smega[:, 128 * h:128 * h + 128], ks, qs,
                                     start=True, stop=False,
                                     tile_position=tp, skip_group_check=True)
                st.update(qv=qv, off=off, smega=smega, g=g)

            def stage_red(g, st):
                """Pool: negated per-(head,q) colmax of S^T via partition reduce."""
                smega = st["smega"]
                negm = sbsm.tile([1, 1024], F16, tag="negm")
                nc.gpsimd.tensor_reduce(
                    negm[:, :], smega[:, :],
                    axis=mybir.AxisListType.C, op=mybir.AluOpType.max,
                    negate=True)
                st["negm"] = negm

            def stage_msub(g, st):
                """PE: K=1 matmul broadcasts -rowmax into every S^T row."""
                nc.tensor.matmul(st["smega"][:, :], ones1_sb[:, :],
                                 st.pop("negm")[:, :],
                                 start=False, stop=True, skip_group_check=True)

            def stage_exp(g, st):
                """Act: single unbiased exp; output IS E^T (k-major)."""
                esb = sbe.tile([128, 1024], F16, tag="esb")
                nc.scalar.activation(esb[:, :], st["smega"][:, :], EXP,
                                     bias=0.0, scale=1.0)
                st["etsb"] = esb

            def stage_y(g, st):
                qv, off, etsb = st["qv"], st["off"], st["etsb"]
                yp = psY.tile([128, 264], F32, tag="y")
                for h in range(8):
                    nc.tensor.matmul(
                        yp[:, 33 * h:33 * h + 33],
                        etsb[:, 128 * h:128 * h + 128],
                        qv[:, off + 512 + 33 * h: off + 512 + 33 * h + 33],
                        start=True, stop=True)
                rinv = sbsm.tile([128, 8], F32, tag="rinv")
                y3 = yp.rearrange("p (h j) -> p h j", h=8)
                nc.vector.reciprocal(rinv.rearrange("p (h j) -> p h j", j=1),
                                     y3[:, :, 32:33])
                st.update(yp=yp, rinv=rinv)

            def stage_ynorm(g, st):
                yp, rinv = st.pop("yp"), st.pop("rinv")
                y3 = yp.rearrange("p (h j) -> p h j", h=8)
                ysb = sbsm.tile([128, 256], F16, tag="ysb")
                _eng(nc, YNORM_ENGINE).tensor_tensor(
                    ysb.rearrange("p (h j) -> p h j", h=8),
                    y3[:, :, 0:32],
                    rinv.rearrange("p (h j) -> p h j", j=1).broadcast_to([128, 8, 32]),
                    op=mybir.AluOpType.mult)
                st["ysb"] = ysb

            def stage_yt(g, st):
                ysb = st.pop("ysb")
                ytp = psO.tile([128, 256], F16, tag="o", name="ytp")
                for ck in range(2):
                    nc.tensor.transpose(ytp[:, 128 * ck:128 * ck + 128],
                                        ysb[:, 128 * ck:128 * ck + 128], idm[:, :])
                ytsb = sbsm.tile([128, 256], F16, tag="ytsb")
                _copy(nc, YTCOPY_ENGINE, ytsb[:, :], ytp[:, :])
                st["ytsb"] = ytsb

            def stage_proj(g, st):
                ytsb = st.pop("ytsb")
                op = psO.tile([128, C], F32, tag="o")
                if with_bias:
                    nc.tensor.matmul(op[:, :], ones1_sb[:, :], bias_sb[:, :],
                                     start=True, stop=False)
                nc.tensor.matmul(op[:, :], ytsb[:, 0:128], wt_sb[:, 0:C],
                                 start=not with_bias, stop=False)
                nc.tensor.matmul(op[:, :], ytsb[:, 128:256], wt_sb[:, C:2 * C],
                                 start=False, stop=True)
                if g % GB == 0:
                    shared["osb"] = sbo.tile([128, GB * C], F16, tag="osb",
                                             name="osb")
                osb = shared["osb"]
                gi = g % GB
                _copy(nc, OUTCOPY_ENGINE, osb[:, C * gi:C * gi + C], op[:, :])
                if gi == GB - 1:
                    nc.sync.dma_start(
                        outd[g // GB, :, :, :].rearrange("g p j -> p g j"),
                        osb.rearrange("p (g j) -> p g j", g=GB))

            fetch(0)
            sts = {}
            LAG = 4
            for it in range(NG + LAG):
                def live(k):
                    return 0 <= k < NG

                if live(it):
                    sts[it] = {}
                    stage_s(it, sts[it])                 # PE: S first
                if live(it - 1):
                    stage_msub(it - 1, sts[it - 1])      # PE
                if live(it - 4):
                    stage_proj(it - 4, sts[it - 4])      # PE; outcopy
                if live(it):
                    stage_red(it, sts[it])               # Pool: red
                if live(it - 1):
                    stage_exp(it - 1, sts[it - 1])       # Act
                if live(it - 2):
                    stage_y(it - 2, sts[it - 2])         # PE Y; DVE recip
                    stage_ynorm(it - 2, sts[it - 2])     # DVE/Pool
                if live(it - 3):
                    stage_yt(it - 3, sts[it - 3])        # PE YT; YTcopy
                if live(it - 4):
                    sts.pop(it - 4)

    nc.finalize()
    return nc


def _prep_inputs(qkv, sim, proj_w, proj_b, scale):
    """Host-side shard + pack. Returns (in_maps, sort_indices, with_bias)."""
    b, n, _ = qkv.shape
    tk = np.argmax(sim, axis=-1)
    sort_idx = np.argsort(tk, axis=-1, kind="stable")

    wt_full = np.ascontiguousarray(proj_w.T).astype(np.float16)   # [cin, cout]
    with_bias = bool(np.any(proj_b != 0))
    bias16 = proj_b.reshape(1, C).astype(np.float16)
    idm = np.eye(128, dtype=np.float16)

    in_maps = []
    for cx in range(8):
        bi, half = cx // 2, cx % 2
        perm = sort_idx[bi, 8192 * half:8192 * (half + 1)]
        shuf = qkv[bi][perm].astype(np.float32)                    # [8192, 768]
        qk = shuf[:, 0:512].copy()
        qk[:, 0:256] *= scale
        # [g, tok, 4, 128] -> [g, p=ch, c, tok]
        qkt = qk.astype(np.float16).reshape(NG, 128, 4, 128).transpose(0, 3, 2, 1)
        qkt = qkt.reshape(NG, 128, 512)
        vpart = np.empty((NG, 128, 8, 33), dtype=np.float16)
        vpart[:, :, :, 0:32] = shuf[:, 512:768].reshape(NG, 128, 8, 32)
        vpart[:, :, :, 32] = 1.0
        qv = np.concatenate([qkt, vpart.reshape(NG, 128, 264)], axis=2)
        qv = np.ascontiguousarray(qv.reshape(NG // GB, GB, 128, W))
        m = {"qvd": qv, "wtd": wt_full, "idmd": idm}
        if with_bias:
            m["biasd"] = bias16
        in_maps.append(m)
    return in_maps, sort_idx, with_bias


def kernel(qkv, sim, proj_w, proj_b, logit_scale, h=128, w=128, **_unused):
    qkv = np.ascontiguousarray(np.asarray(qkv, dtype=np.float32))
    sim = np.asarray(sim, dtype=np.float32)
    proj_w = np.asarray(proj_w, dtype=np.float32)
    proj_b = np.asarray(proj_b, dtype=np.float32)
    ls = float(np.asarray(logit_scale, dtype=np.float32).reshape(-1)[0])
    scale = float(np.exp(min(ls, float(np.log(100.0)))))

    b, n, c3 = qkv.shape
    assert (b, n, c3) == (4, 16384, 768)

    in_maps, sort_idx, with_bias = _prep_inputs(qkv, sim, proj_w, proj_b, scale)

    key = ("b" if with_bias else "nb")
    if key not in _cache:
        _cache[key] = _build(with_bias)
    nc = _cache[key]

    global _last_in_maps
    _last_in_maps = in_maps
    res = run_bass_kernel_spmd(nc, in_maps, core_ids=list(range(8)))

    outf = np.empty((4, 16384, 256), dtype=np.float32)
    for cx in range(8):
        bi, half = cx // 2, cx % 2
        perm = sort_idx[bi, 8192 * half:8192 * (half + 1)]
        y = np.asarray(res.results[cx]["outd"]).astype(np.float32).reshape(8192, 256)
        outf[bi][perm] = y
    return outf


if __name__ == "__main__":
    rng = np.random.default_rng(0)
    qkv = rng.standard_normal((4, 16384, 768), dtype=np.float32)
    sim = rng.standard_normal((4, 16384, 64), dtype=np.float32)
    pw = (rng.standard_normal((256, 256), dtype=np.float32) * 0.02)
    pb = np.zeros(256, dtype=np.float32)
    lsc = np.log(10.0 * np.ones((1, 1), dtype=np.float32))
    o = kernel(qkv=qkv, sim=sim, proj_w=pw, proj_b=pb, logit_scale=lsc)
    print("ran", o.shape, o.dtype)
